# revision 16
# baseline (speedup 1.0000x reference)
"""Trainium2 Bass kernel for CorrelationModule (per-pixel self-attention).

Math (per batch element b, all fp32):
  xf = x[b] reshaped [C=384, N=2304]
  q = Wq@xf + bq, k = Wk@xf + bk                       (1x1 convs)
  attn = softmax_m(q^T k / sqrt(512))                  (N x N)
  out = Wo @ ((Wv@xf + bv) @ attn^T) + bo              -> [512, N]

Sharding: batch B=8 data-parallel across the 8 NeuronCores, params replicated.

Per-core kernel layout choices:
  - Wo is folded into V on the host: out = (Wo@Wv@xf) @ attn^T + (Wo@bv+bo)
    (valid because attn rows sum to 1), which deletes the whole output-
    projection matmul phase.
  - Scores are computed TRANSPOSED: s_t[m, n] = sum_o k[o,m] q[o,n], so the
    softmax reduction (over m) lands on the partition axis.
  - exp is taken without max-subtraction: scores*scale ~ N(0, 1/9), so
    exp() cannot overflow for this module's data distribution.
  - Softmax normalization is deferred: AV runs on the unnormalized
    exp-scores; the final tile is multiplied by the broadcast reciprocal
    row sums.  The row sums come from a ones-matmul partition reduction
    (TensorE, ~0.2us) instead of a gpsimd all-reduce (~3.2us).
  - q/k and v/exp-scores are stored as fp8e4 pairs [128, 2, free] so the
    big NxN matmuls run in DoubleRow perf mode (2 fp8 weights per PE
    cell; measured 216ns per [256 x 128, 1024] matmul = full 2x bf16).
    PSUM accumulation stays fp32.
  - Projection matmul operands are fp16 (1 row/cycle PE rate).
  - The per-block epilogue is split: av-draining muls (DVE) run early so
    the next block's AV matmuls get their PSUM banks back; the bias-add +
    store (ACT FIFO) is emitted after the next block's first exps so it
    never head-of-line-blocks them.
"""

import numpy as np

B, C, O, H, W = 8, 384, 512, 48, 48
N = H * W  # 2304 tokens
P = 128
CT, OT, MT = C // P, O // P, N // P  # 3, 4, 18
OP, MP = OT // 2, MT // 2  # o-pairs, m-pairs for fp8 DoubleRow
NBLK = [(0, 512), (512, 512), (1024, 512), (1536, 512), (2048, 256)]
SCALE = 1.0 / float(np.sqrt(O))

_cache = {}


def _build_nc():
    import concourse.bacc as bacc
    import concourse.tile as tile
    import concourse.mybir as mybir

    F32 = mybir.dt.float32
    F16 = mybir.dt.float16
    F8 = mybir.dt.float8e4
    DR = mybir.MatmulPerfMode.DoubleRow

    nc = bacc.Bacc(
        "TRN2",
        target_bir_lowering=False,
        debug=False,
        enable_asserts=False,
        num_devices=1,
    )

    xf_d = nc.dram_tensor("xf", [C, N], F16, kind="ExternalInput").ap()
    wqkv_d = nc.dram_tensor("wqkv", [C, 3 * O], F16, kind="ExternalInput").ap()
    bias_d = nc.dram_tensor("bias", [O, 3], F32, kind="ExternalInput").ap()
    y_d = nc.dram_tensor("y", [O, N], F32, kind="ExternalOutput").ap()

    with tile.TileContext(nc) as tc:
        with (
            nc.allow_low_precision(reason="fp16/fp8 matmul operands"),
            tc.tile_pool(name="const", bufs=1) as const,
            tc.tile_pool(name="work", bufs=1) as work,
            tc.tile_pool(name="ps", bufs=1, space="PSUM") as ps,
        ):
            # ---- persistent SBUF tensors -------------------------------
            xf_sb = [
                const.tile([P, N], F16, tag=f"xf{c}", name=f"xf_sb{c}")
                for c in range(CT)
            ]
            wqkv_sb = [
                const.tile([P, 3 * O], F16, tag=f"wqkv{c}", name=f"wqkv_sb{c}")
                for c in range(CT)
            ]
            wqt_sb = [t[:, 0:O] for t in wqkv_sb]
            wkt_sb = [t[:, O:2 * O] for t in wqkv_sb]
            wvt_sb = [t[:, 2 * O:3 * O] for t in wqkv_sb]
            bias_sb = const.tile([P, OT, 3], F32, tag="bias", name="bias_sb")
            bq_sb = [bias_sb[:, o, 0:1] for o in range(OT)]
            bk_sb = [bias_sb[:, o, 1:2] for o in range(OT)]
            bo2_sb = [bias_sb[:, o, 2:3] for o in range(OT)]
            F32R = mybir.dt.float32r
            ones_f32 = const.tile([P, P], F32, tag="ones32", name="ones_f32")
            ones_sb = const.tile([P, P], F32R, tag="ones", name="ones_sb")
            nc.vector.memset(ones_f32[:], 1.0)
            nc.vector.tensor_copy(ones_sb[:], ones_f32[:])

            # K as fp8 o-pairs for DoubleRow: k_sb[p][:, j, m] = o-tile 2p+j
            k_sb = [
                const.tile([P, 2, N], F8, tag=f"k{p}", name=f"k_sb{p}")
                for p in range(OP)
            ]
            # V' as fp8 m-pairs: vt_sb[t][:, j, o] = m-tile 2t+j
            vt_sb = [
                const.tile([P, 2, O], F8, tag=f"vt{t}", name=f"vt_sb{t}")
                for t in range(MP)
            ]

            # input DMAs: per-descriptor ISSUE cost on the ring engine is
            # ~650ns and the scalar/ACT ring exits the framework prologue
            # late, so only sync + gpsimd carry inputs.  First-matmul gate:
            # wk (sync head) + xf chunk 0 (gpsimd head), both landed by
            # ~1.5us after the rings start.
            for c in range(CT):
                nc.sync.dma_start(wqkv_sb[c][:, O:2 * O],
                                  wqkv_d[c * P:(c + 1) * P, O:2 * O])
            for c in range(CT):
                nc.gpsimd.dma_start(xf_sb[c][:, 0:512],
                                    xf_d[c * P:(c + 1) * P, 0:512])
            nc.gpsimd.dma_start(bias_sb[:],
                                bias_d.rearrange("(t p) c -> p t c", p=P))
            for c in range(CT):
                nc.sync.dma_start(xf_sb[c][:, 512:1536],
                                  xf_d[c * P:(c + 1) * P, 512:1536])
            for c in range(CT):
                nc.gpsimd.dma_start(wqkv_sb[c][:, 2 * O:3 * O],
                                    wqkv_d[c * P:(c + 1) * P, 2 * O:3 * O])
            for c in range(CT):
                nc.sync.dma_start(xf_sb[c][:, 1536:N],
                                  xf_d[c * P:(c + 1) * P, 1536:N])
            for c in range(CT):
                nc.gpsimd.dma_start(wqkv_sb[c][:, 0:O],
                                    wqkv_d[c * P:(c + 1) * P, 0:O])

            # ---- phase 1: K = Wk@xf + bk  (layout [o, m]) --------------
            for n0, nw in NBLK:
                for o in range(OT):
                    osl = slice(o * P, (o + 1) * P)
                    kp = ps.tile([P, nw], F32, tag="s", bufs=4, name=f"kp_{o}_{n0}")
                    for c in range(CT):
                        nc.tensor.matmul(
                            kp[:],
                            wkt_sb[c][:, osl],
                            xf_sb[c][:, n0:n0 + nw],
                            start=(c == 0),
                            stop=(c == CT - 1),
                        )
                    nc.scalar.add(k_sb[o // 2][:, o % 2, n0:n0 + nw], kp[:],
                                  bk_sb[o][:])

            # ---- phase 1b: V'^T = ((Wo@Wv)@xf)^T  (layout [m, o]) ------
            for m in range(MT):
                msl = slice(m * P, (m + 1) * P)
                vp = ps.tile([P, O], F32, tag="s", bufs=4, name=f"vp_{m}")
                for c in range(CT):
                    nc.tensor.matmul(
                        vp[:],
                        xf_sb[c][:, msl],
                        wvt_sb[c][:],
                        start=(c == 0),
                        stop=(c == CT - 1),
                    )
                nc.vector.tensor_copy(vt_sb[m // 2][:, m % 2, :], vp[:])

            # ---- phase 2: flash attention over n-blocks ----------------
            finish_early = None  # prev block: denom + av-drain muls (PE/DVE)
            finish_late = None   # prev block: bias-add + store (ACT/sync)
            for n0, nw in NBLK:
                nsl = slice(n0, n0 + nw)
                # Q for this block (fp8 o-pairs), bias bq added on DVE so
                # the ACT queue stays free for the exps
                q_sb = [
                    work.tile([P, 2, nw], F8, tag=f"q{p}", bufs=2,
                              name=f"q_{n0}_{p}")
                    for p in range(OP)
                ]
                for o in range(OT):
                    osl = slice(o * P, (o + 1) * P)
                    qp = ps.tile([P, nw], F32, tag="s", bufs=4, name=f"qp_{n0}_{o}")
                    for c in range(CT):
                        nc.tensor.matmul(
                            qp[:],
                            wqt_sb[c][:, osl],
                            xf_sb[c][:, nsl],
                            start=(c == 0),
                            stop=(c == CT - 1),
                        )
                    nc.scalar.add(q_sb[o // 2][:, o % 2, :], qp[:], bq_sb[o][:])

                if finish_early is not None:
                    finish_early()
                    finish_early = None

                av_ps = [
                    ps.tile([P, nw], F32, tag=f"av{o}", bufs=1,
                            name=f"av_{n0}_{o}")
                    for o in range(OT)
                ]
                eacc = work.tile([P, nw], F32R, tag="eacc", bufs=2,
                                 name=f"eacc_{n0}")

                for t in range(MP):
                    e8 = work.tile([P, 2, nw], F8, tag="e", bufs=3,
                                   name=f"e_{n0}_{t}")
                    for j, m in enumerate((2 * t, 2 * t + 1)):
                        msl = slice(m * P, (m + 1) * P)
                        sp = ps.tile([P, nw], F32, tag="s", bufs=4,
                                     name=f"sp_{n0}_{m}")
                        for p in range(OP):
                            nc.tensor.matmul(
                                sp[:],
                                k_sb[p][:, :, msl],
                                q_sb[p][:],
                                start=(p == 0),
                                stop=(p == OP - 1),
                                perf_mode=DR,
                            )
                        nc.scalar.activation(
                            e8[:, j, :], sp[:],
                            mybir.ActivationFunctionType.Exp,
                            scale=SCALE,
                        )
                        if t == 0 and j == 0:
                            nc.vector.tensor_copy(eacc[:], e8[:, 0, :])
                        else:
                            nc.vector.tensor_add(eacc[:], eacc[:], e8[:, j, :])
                    for o in range(OT):
                        osl = slice(o * P, (o + 1) * P)
                        nc.tensor.matmul(
                            av_ps[o][:],
                            vt_sb[t][:, :, osl],
                            e8[:],
                            start=(t == 0),
                            stop=(t == MP - 1),
                            perf_mode=DR,
                        )
                    if t == 0 and finish_late is not None:
                        finish_late()
                        finish_late = None

                # denominator at the block tail: the fp32r ones-matmul
                # all-reduces eacc over partitions (full rate at free dim
                # >= 256, no cast op needed; the final eacc add lands
                # during the last AV matmuls so the PE barely waits), then
                # the fast reciprocal gives the broadcast 1/denom tile.
                dp = ps.tile([P, nw], F32, tag="s", bufs=4, name=f"dp_{n0}")
                nc.tensor.matmul(dp[:], ones_sb[:], eacc[:])
                rb = work.tile([P, nw], F32, tag="rb_sb", bufs=2,
                               name=f"rb_{n0}")
                nc.vector.reciprocal_approx_fast(out=rb[:], in_=dp[:])

                def make_finishes(n0=n0, nw=nw, nsl=nsl, av_ps=av_ps, rb=rb):
                    tmps = []

                    def early():
                        # rb is already done, so these muls drain the av
                        # PSUM banks with no latency chain at the boundary
                        for o in range(OT):
                            tmp = work.tile([P, nw], F32, tag="tmp", bufs=8,
                                            name=f"tmp_{n0}_{o}")
                            nc.vector.tensor_mul(tmp[:], av_ps[o][:], rb[:])
                            tmps.append(tmp)

                    def late():
                        for o in range(OT):
                            osl = slice(o * P, (o + 1) * P)
                            outt = work.tile([P, nw], F32, tag="out", bufs=8,
                                             name=f"out_{n0}_{o}")
                            nc.scalar.add(outt[:], tmps[o][:], bo2_sb[o][:])
                            nc.sync.dma_start(y_d[osl, nsl], outt[:])
                    return early, late

                finish_early, finish_late = make_finishes()

            finish_early()
            finish_late()

    nc.compile()
    return nc


def get_nc():
    if "nc" not in _cache:
        _cache["nc"] = _build_nc()
    return _cache["nc"]


def make_in_maps(x, Wq, bq, Wk, bk, Wv, bv, Wo, bo):
    x = np.asarray(x, np.float32)
    Wq = np.asarray(Wq, np.float32)
    Wk = np.asarray(Wk, np.float32)
    Wv = np.asarray(Wv, np.float32)
    Wo = np.asarray(Wo, np.float32)
    bq = np.asarray(bq, np.float32)
    bk = np.asarray(bk, np.float32)
    bv = np.asarray(bv, np.float32)
    bo = np.asarray(bo, np.float32)

    Wvo = Wo @ Wv  # fold the output projection into V
    wqkv = np.concatenate([Wq.T, Wk.T, Wvo.T], axis=1).astype(np.float16)
    bo2 = (Wo @ bv + bo).astype(np.float32)
    bias = np.stack([bq, bk, bo2], axis=1).astype(np.float32)

    xf = x.reshape(B, C, N).astype(np.float16)
    shared = {
        "wqkv": np.ascontiguousarray(wqkv),
        "bias": np.ascontiguousarray(bias),
    }
    return [
        {"xf": np.ascontiguousarray(xf[b]), **shared} for b in range(B)
    ]


def kernel(x, Wq, bq, Wk, bk, Wv, bv, Wo, bo):
    from concourse import bass_utils

    nc = get_nc()
    in_maps = make_in_maps(x, Wq, bq, Wk, bk, Wv, bv, Wo, bo)
    res = bass_utils.run_bass_kernel_spmd(nc, in_maps, core_ids=list(range(B)))
    y = np.stack([res.results[b]["y"] for b in range(B)], axis=0)
    return np.ascontiguousarray(y.reshape(B, O, H, W))


# revision 17
# speedup vs baseline: 1.0134x; 1.0134x over previous
"""Trainium2 Bass kernel for CorrelationModule (per-pixel self-attention).

Math (per batch element b, all fp32):
  xf = x[b] reshaped [C=384, N=2304]
  q = Wq@xf + bq, k = Wk@xf + bk                       (1x1 convs)
  attn = softmax_m(q^T k / sqrt(512))                  (N x N)
  out = Wo @ ((Wv@xf + bv) @ attn^T) + bo              -> [512, N]

Sharding: batch B=8 data-parallel across the 8 NeuronCores, params replicated.

Per-core kernel layout choices:
  - Wo is folded into V on the host: out = (Wo@Wv@xf) @ attn^T + (Wo@bv+bo)
    (valid because attn rows sum to 1), which deletes the whole output-
    projection matmul phase.
  - Scores are computed TRANSPOSED: s_t[m, n] = sum_o k[o,m] q[o,n], so the
    softmax reduction (over m) lands on the partition axis.
  - exp is taken without max-subtraction: scores*scale ~ N(0, 1/9), so
    exp() cannot overflow for this module's data distribution.
  - Softmax normalization is deferred: AV runs on the unnormalized
    exp-scores; the final tile is multiplied by the broadcast reciprocal
    row sums.  The row sums come from a ones-matmul partition reduction
    (TensorE, ~0.2us) instead of a gpsimd all-reduce (~3.2us).
  - q/k and v/exp-scores are stored as fp8e4 pairs [128, 2, free] so the
    big NxN matmuls run in DoubleRow perf mode (2 fp8 weights per PE
    cell; measured 216ns per [256 x 128, 1024] matmul = full 2x bf16).
    PSUM accumulation stays fp32.
  - Projection matmul operands are fp16 (1 row/cycle PE rate).
  - The per-block epilogue is split: av-draining muls (DVE) run early so
    the next block's AV matmuls get their PSUM banks back; the bias-add +
    store (ACT FIFO) is emitted after the next block's first exps so it
    never head-of-line-blocks them.
"""

import numpy as np

B, C, O, H, W = 8, 384, 512, 48, 48
N = H * W  # 2304 tokens
P = 128
CT, OT, MT = C // P, O // P, N // P  # 3, 4, 18
OP, MP = OT // 2, MT // 2  # o-pairs, m-pairs for fp8 DoubleRow
NBLK = [(0, 512), (512, 512), (1024, 512), (1536, 512), (2048, 256)]
SCALE = 1.0 / float(np.sqrt(O))

_cache = {}


def _build_nc():
    import concourse.bacc as bacc
    import concourse.tile as tile
    import concourse.mybir as mybir

    F32 = mybir.dt.float32
    F16 = mybir.dt.float16
    F8 = mybir.dt.float8e4
    DR = mybir.MatmulPerfMode.DoubleRow

    nc = bacc.Bacc(
        "TRN2",
        target_bir_lowering=False,
        debug=False,
        enable_asserts=False,
        num_devices=1,
    )

    xf_d = nc.dram_tensor("xf", [C, N], F16, kind="ExternalInput").ap()
    wqkv_d = nc.dram_tensor("wqkv", [C, 3 * O], F16, kind="ExternalInput").ap()
    bias_d = nc.dram_tensor("bias", [O, 3], F32, kind="ExternalInput").ap()
    y_d = nc.dram_tensor("y", [O, N], F32, kind="ExternalOutput").ap()

    with tile.TileContext(nc) as tc:
        with (
            nc.allow_low_precision(reason="fp16/fp8 matmul operands"),
            tc.tile_pool(name="const", bufs=1) as const,
            tc.tile_pool(name="work", bufs=1) as work,
            tc.tile_pool(name="ps", bufs=1, space="PSUM") as ps,
        ):
            # ---- persistent SBUF tensors -------------------------------
            xf_sb = [
                const.tile([P, N], F16, tag=f"xf{c}", name=f"xf_sb{c}")
                for c in range(CT)
            ]
            wqkv_sb = [
                const.tile([P, 3 * O], F16, tag=f"wqkv{c}", name=f"wqkv_sb{c}")
                for c in range(CT)
            ]
            wqt_sb = [t[:, 0:O] for t in wqkv_sb]
            wkt_sb = [t[:, O:2 * O] for t in wqkv_sb]
            wvt_sb = [t[:, 2 * O:3 * O] for t in wqkv_sb]
            bias_sb = const.tile([P, OT, 3], F32, tag="bias", name="bias_sb")
            bq_sb = [bias_sb[:, o, 0:1] for o in range(OT)]
            bk_sb = [bias_sb[:, o, 1:2] for o in range(OT)]
            bo2_sb = [bias_sb[:, o, 2:3] for o in range(OT)]
            ones_sb = const.tile([P, P], F16, tag="ones", name="ones_sb")
            nc.vector.memset(ones_sb[:], 1.0)
            junk_sb = const.tile([P, P], F16, tag="junk", name="junk_sb")
            nc.vector.memset(junk_sb[:], 0.0)

            # K as fp8 o-pairs for DoubleRow: k_sb[p][:, j, m] = o-tile 2p+j
            k_sb = [
                const.tile([P, 2, N], F8, tag=f"k{p}", name=f"k_sb{p}")
                for p in range(OP)
            ]
            # V' as fp8 m-pairs: vt_sb[t][:, j, o] = m-tile 2t+j
            vt_sb = [
                const.tile([P, 2, O], F8, tag=f"vt{t}", name=f"vt_sb{t}")
                for t in range(MP)
            ]

            # input DMAs: per-descriptor ISSUE cost on the ring engine is
            # ~650ns and the scalar/ACT ring exits the framework prologue
            # late, so only sync + gpsimd carry inputs.  First-matmul gate:
            # wk (sync head) + xf chunk 0 (gpsimd head), both landed by
            # ~1.5us after the rings start.
            for c in range(CT):
                nc.sync.dma_start(wqkv_sb[c][:, O:2 * O],
                                  wqkv_d[c * P:(c + 1) * P, O:2 * O])
            for c in range(CT):
                nc.gpsimd.dma_start(xf_sb[c][:, 0:512],
                                    xf_d[c * P:(c + 1) * P, 0:512])
            nc.gpsimd.dma_start(bias_sb[:],
                                bias_d.rearrange("(t p) c -> p t c", p=P))
            for c in range(CT):
                nc.sync.dma_start(xf_sb[c][:, 512:1536],
                                  xf_d[c * P:(c + 1) * P, 512:1536])
            for c in range(CT):
                nc.gpsimd.dma_start(wqkv_sb[c][:, 2 * O:3 * O],
                                    wqkv_d[c * P:(c + 1) * P, 2 * O:3 * O])
            for c in range(CT):
                nc.sync.dma_start(xf_sb[c][:, 1536:N],
                                  xf_d[c * P:(c + 1) * P, 1536:N])
            for c in range(CT):
                nc.gpsimd.dma_start(wqkv_sb[c][:, 0:O],
                                    wqkv_d[c * P:(c + 1) * P, 0:O])

            # PE warm-up: ~20 dependency-free short matmuls on a zeroed
            # tile run while the first input DMAs land, so the HAM clock
            # gate is already at 8/8 when the real matmuls start.
            warm = ps.tile([P, P], F32, tag="av0", bufs=1, name="warm_ps")
            for i in range(20):
                nc.tensor.matmul(warm[:], junk_sb[:], junk_sb[:])

            # ---- phase 1: K = Wk@xf + bk  (layout [o, m]) --------------
            for n0, nw in NBLK:
                for o in range(OT):
                    osl = slice(o * P, (o + 1) * P)
                    kp = ps.tile([P, nw], F32, tag="s", bufs=4, name=f"kp_{o}_{n0}")
                    for c in range(CT):
                        nc.tensor.matmul(
                            kp[:],
                            wkt_sb[c][:, osl],
                            xf_sb[c][:, n0:n0 + nw],
                            start=(c == 0),
                            stop=(c == CT - 1),
                        )
                    nc.scalar.add(k_sb[o // 2][:, o % 2, n0:n0 + nw], kp[:],
                                  bk_sb[o][:])

            # ---- phase 1b: V'^T = ((Wo@Wv)@xf)^T  (layout [m, o]) ------
            for m in range(MT):
                msl = slice(m * P, (m + 1) * P)
                vp = ps.tile([P, O], F32, tag="s", bufs=4, name=f"vp_{m}")
                for c in range(CT):
                    nc.tensor.matmul(
                        vp[:],
                        xf_sb[c][:, msl],
                        wvt_sb[c][:],
                        start=(c == 0),
                        stop=(c == CT - 1),
                    )
                nc.vector.tensor_copy(vt_sb[m // 2][:, m % 2, :], vp[:])

            # ---- phase 2: flash attention over n-blocks ----------------
            finish_early = None  # prev block: denom + av-drain muls (PE/DVE)
            finish_late = None   # prev block: bias-add + store (ACT/sync)
            for n0, nw in NBLK:
                nsl = slice(n0, n0 + nw)
                # Q for this block (fp8 o-pairs), bias bq added on DVE so
                # the ACT queue stays free for the exps
                q_sb = [
                    work.tile([P, 2, nw], F8, tag=f"q{p}", bufs=2,
                              name=f"q_{n0}_{p}")
                    for p in range(OP)
                ]
                for o in range(OT):
                    osl = slice(o * P, (o + 1) * P)
                    qp = ps.tile([P, nw], F32, tag="s", bufs=4, name=f"qp_{n0}_{o}")
                    for c in range(CT):
                        nc.tensor.matmul(
                            qp[:],
                            wqt_sb[c][:, osl],
                            xf_sb[c][:, nsl],
                            start=(c == 0),
                            stop=(c == CT - 1),
                        )
                    nc.scalar.add(q_sb[o // 2][:, o % 2, :], qp[:], bq_sb[o][:])

                if finish_early is not None:
                    finish_early()
                    finish_early = None

                av_ps = [
                    ps.tile([P, nw], F32, tag=f"av{o}", bufs=1,
                            name=f"av_{n0}_{o}")
                    for o in range(OT)
                ]
                eacc = work.tile([P, nw], F32, tag="eacc", bufs=2,
                                 name=f"eacc_{n0}")

                for t in range(MP):
                    e8 = work.tile([P, 2, nw], F8, tag="e", bufs=3,
                                   name=f"e_{n0}_{t}")
                    for j, m in enumerate((2 * t, 2 * t + 1)):
                        msl = slice(m * P, (m + 1) * P)
                        sp = ps.tile([P, nw], F32, tag="s", bufs=4,
                                     name=f"sp_{n0}_{m}")
                        for p in range(OP):
                            nc.tensor.matmul(
                                sp[:],
                                k_sb[p][:, :, msl],
                                q_sb[p][:],
                                start=(p == 0),
                                stop=(p == OP - 1),
                                perf_mode=DR,
                            )
                        nc.scalar.activation(
                            e8[:, j, :], sp[:],
                            mybir.ActivationFunctionType.Exp,
                            scale=SCALE,
                        )
                        if t == 0 and j == 0:
                            nc.vector.tensor_copy(eacc[:], e8[:, 0, :])
                        else:
                            nc.vector.tensor_add(eacc[:], eacc[:], e8[:, j, :])
                    for o in range(OT):
                        osl = slice(o * P, (o + 1) * P)
                        nc.tensor.matmul(
                            av_ps[o][:],
                            vt_sb[t][:, :, osl],
                            e8[:],
                            start=(t == 0),
                            stop=(t == MP - 1),
                            perf_mode=DR,
                        )
                    if t == 0 and finish_late is not None:
                        finish_late()
                        finish_late = None

                # block tail: cast the accumulated exp to fp16 for the
                # ones-matmul (the cast lands during the last AV matmuls)
                e16 = work.tile([P, nw], F16, tag="e16", bufs=2,
                                name=f"e16_{n0}")
                nc.vector.tensor_copy(e16[:], eacc[:])

                def make_finishes(n0=n0, nw=nw, nsl=nsl, av_ps=av_ps,
                                  e16=e16):
                    tmps = []

                    def early():
                        # ones-matmul all-reduces the exp sums over
                        # partitions (every partition gets the column
                        # sums), fast reciprocal gives the broadcast
                        # 1/denom, and the muls drain the av PSUM banks
                        # for the next block's AV matmuls.
                        dp = ps.tile([P, nw], F32, tag="s", bufs=4,
                                     name=f"dp_{n0}")
                        nc.tensor.matmul(dp[:], ones_sb[:], e16[:])
                        rb = work.tile([P, nw], F32, tag="rb_sb", bufs=2,
                                       name=f"rb_{n0}")
                        nc.vector.reciprocal_approx_fast(out=rb[:], in_=dp[:])
                        for o in range(OT):
                            tmp = work.tile([P, nw], F32, tag="tmp", bufs=8,
                                            name=f"tmp_{n0}_{o}")
                            nc.vector.tensor_mul(tmp[:], av_ps[o][:], rb[:])
                            tmps.append(tmp)

                    def late():
                        for o in range(OT):
                            osl = slice(o * P, (o + 1) * P)
                            outt = work.tile([P, nw], F32, tag="out", bufs=8,
                                             name=f"out_{n0}_{o}")
                            nc.scalar.add(outt[:], tmps[o][:], bo2_sb[o][:])
                            nc.sync.dma_start(y_d[osl, nsl], outt[:])
                    return early, late

                finish_early, finish_late = make_finishes()

            finish_early()
            finish_late()

    nc.compile()
    return nc


def get_nc():
    if "nc" not in _cache:
        _cache["nc"] = _build_nc()
    return _cache["nc"]


def make_in_maps(x, Wq, bq, Wk, bk, Wv, bv, Wo, bo):
    x = np.asarray(x, np.float32)
    Wq = np.asarray(Wq, np.float32)
    Wk = np.asarray(Wk, np.float32)
    Wv = np.asarray(Wv, np.float32)
    Wo = np.asarray(Wo, np.float32)
    bq = np.asarray(bq, np.float32)
    bk = np.asarray(bk, np.float32)
    bv = np.asarray(bv, np.float32)
    bo = np.asarray(bo, np.float32)

    Wvo = Wo @ Wv  # fold the output projection into V
    wqkv = np.concatenate([Wq.T, Wk.T, Wvo.T], axis=1).astype(np.float16)
    bo2 = (Wo @ bv + bo).astype(np.float32)
    bias = np.stack([bq, bk, bo2], axis=1).astype(np.float32)

    xf = x.reshape(B, C, N).astype(np.float16)
    shared = {
        "wqkv": np.ascontiguousarray(wqkv),
        "bias": np.ascontiguousarray(bias),
    }
    return [
        {"xf": np.ascontiguousarray(xf[b]), **shared} for b in range(B)
    ]


def kernel(x, Wq, bq, Wk, bk, Wv, bv, Wo, bo):
    from concourse import bass_utils

    nc = get_nc()
    in_maps = make_in_maps(x, Wq, bq, Wk, bk, Wv, bv, Wo, bo)
    res = bass_utils.run_bass_kernel_spmd(nc, in_maps, core_ids=list(range(B)))
    y = np.stack([res.results[b]["y"] for b in range(B)], axis=0)
    return np.ascontiguousarray(y.reshape(B, O, H, W))
